# revision 54
# baseline (speedup 1.0000x reference)
"""Multi-head attention (B=2, S=2048, H=1024, 16 heads) on 8 TRN2 NeuronCores.

Sharding: data-parallel over batch (2) x tensor-parallel over heads (16 -> 4
groups of 4 heads).  Core c = b*4 + g handles batch b, heads [4g, 4g+4).

Per-core math (fp16 storage / fp32 accumulate), with x = q|k|v of its batch:
  QP_T[d, s] = (Wq_g x^T + bq_g)   stored transposed, d on partitions
  KP_T[d, s] = (Wk_g x^T + bk_g)
  VP[s, d]   = x Wv_g^T            natural layout  (bv folded on host)
  per head h:  S_T[j, i] = KP_T_h^T-contracted scores (d contracts)
               A = exp(S_T / 8)                       (softmax w/o max-sub)
               O_T[d, i] = VP_h^T A   and  L[i] = ones^T A  (PE broadcast)
               O_norm = O_T * (1/L)   1/L = exp(-ln L) on ScalarE, per head
  out_T[o, i] = Wo_g^T-contracted projection of O_norm   -> [1024, 2048] f32
Host: out[b] = sum_g out_T(b,g)^T + (Wo @ bv + bo).
"""

import json

import numpy as np

S = 2048
H = 1024
DL = 256          # local projection dim = 4 heads * 64
P = 128
HD = 64
NK = H // P       # 8 k-tiles over hidden dim
NI = 4            # i blocks of 512 queries
NJ = S // P       # 16 j tiles of 128 keys
NB = 512          # free-dim block

_nc_cache = {}


# --------------------------------------------------------------------------
# BIR fix: this container's walrus supports only ONE sync wait (and update)
# per TPB instruction; Tile attaches several.  Split extras onto single-wait
# EventSemaphore instructions at the serialization boundary.
# --------------------------------------------------------------------------
_wsplit_counter = [0]


def _mk_evsem(engine, debug, wait=None, update=None):
    _wsplit_counter[0] += 1
    return {
        "debug": debug,
        "engine": engine,
        "ins": [],
        "outs": [],
        "name": f"wsplit-{_wsplit_counter[0]}",
        "opcode": "EventSemaphore",
        "sync_info": {
            "on_wait": [wait] if wait else [],
            "on_update": [update] if update else [],
        },
    }


def _split_bir_waits(bir):
    for f in bir.get("functions", []):
        for blk in f.get("blocks", []):
            out = []
            for inst in blk.get("instructions", []):
                si = inst.get("sync_info")
                waits = list(si.get("on_wait") or []) if si else []
                updates = list(si.get("on_update") or []) if si else []
                eng = inst.get("engine")
                dbg = inst.get("debug", 0)
                if len(waits) > 1:
                    for w in waits[:-1]:
                        out.append(_mk_evsem(eng, dbg, wait=w))
                    si["on_wait"] = [waits[-1]]
                out.append(inst)
                if len(updates) > 1:
                    si["on_update"] = [updates[0]]
                    for u in updates[1:]:
                        out.append(_mk_evsem(eng, dbg, update=u))
            blk["instructions"] = out
    return bir


def _install_bir_fix():
    import concourse.bass as bass

    if getattr(bass.Bass, "_wsplit_installed", False):
        return
    orig = bass.Bass.to_json_bytes

    def to_json_bytes(self, *a, **k):
        bir = json.loads(orig(self, *a, **k))
        return json.dumps(_split_bir_waits(bir)).encode()

    bass.Bass.to_json_bytes = to_json_bytes
    bass.Bass._wsplit_installed = True


# --------------------------------------------------------------------------
# Kernel builder
# --------------------------------------------------------------------------

def _build_nc():
    import concourse.bass as bass
    import concourse.mybir as mybir
    import concourse.tile as tile

    f16 = mybir.dt.float16
    f32 = mybir.dt.float32
    Exp = mybir.ActivationFunctionType.Exp
    Ln = mybir.ActivationFunctionType.Ln
    VW = 4 * (HD + 1)  # vp row width: 4 heads x (64 v-dims + 1 ones col)

    nc = bass.Bass("TRN2")

    xq = nc.dram_tensor("xq", [H, S], f16, kind="ExternalInput")
    xk = nc.dram_tensor("xk", [H, S], f16, kind="ExternalInput")
    xv = nc.dram_tensor("xv", [H, S], f16, kind="ExternalInput")
    wqkT = nc.dram_tensor("wqkT", [H, 2 * DL], f16, kind="ExternalInput")
    wvT = nc.dram_tensor("wvT", [H, DL], f16, kind="ExternalInput")
    woT = nc.dram_tensor("woT", [DL, H], f16, kind="ExternalInput")
    bias = nc.dram_tensor("bias", [P, 4], f32, kind="ExternalInput")  # bq0 bq1 bk0 bk1
    out = nc.dram_tensor("out", [H, S], f16, kind="ExternalOutput")

    with tile.TileContext(nc) as tc:
        with (
            tc.tile_pool(name="persist", bufs=1) as persist,
            tc.tile_pool(name="xpool", bufs=8) as xpool,
            tc.tile_pool(name="exppool", bufs=4) as exppool,
            tc.tile_pool(name="lrpool", bufs=2) as lrpool,
            tc.tile_pool(name="evpool", bufs=3) as evpool,
            tc.tile_pool(name="scps", bufs=2, space="PSUM") as scps,
            tc.tile_pool(name="oaob", bufs=1, space="PSUM") as oaob,
            tc.tile_pool(name="ops", bufs=2, space="PSUM") as ops,
        ):
            # ---- persistent tiles ----
            wqk_sb = persist.tile([P, NK, 2 * DL], f16, name="wqk_sb")
            wv_sb = persist.tile([P, NK, DL], f16, name="wv_sb")
            wo_sb = persist.tile([P, 2, H], f16, name="wo_sb")
            bias_sb = persist.tile([P, 4], f32, name="bias_sb")
            onesrow = persist.tile([P, P], f16, name="onesrow")
            qpt = persist.tile([P, 2, S], f16, name="qpt")
            kpt = persist.tile([P, 2, S], f16, name="kpt")
            vp = persist.tile([P, NJ, VW], f16, name="vp")
            onorm = persist.tile([P, 2, S], f16, name="onorm")

            # small constants first (cheap, unblock projections);
            # Wq half before Wk half to match the xq-then-xk load order
            nc.sync.dma_start(bias_sb[:], bias[:])
            wqk_r = wqkT.rearrange("(k p) d -> p k d", p=P)
            nc.sync.dma_start(wqk_sb[:, :, 0:DL], wqk_r[:, :, 0:DL])
            nc.sync.dma_start(wqk_sb[:, :, DL:2 * DL], wqk_r[:, :, DL:2 * DL])

            # ---- x loads ----
            # One grouped DMA per (tensor, part): part 0/1 = 512-col
            # quarters, part 2 = 1024-col half.  Grouping keeps the sync
            # queue's serial DMA-issue cost (~0.65us each) off the
            # critical path.
            xk_t = {}
            xq_t = {}
            xv_t = {}

            def load_group(gd, src, part, tag):
                # part: 0 -> cols 0:512, 1 -> 512:1024, 2 -> 1024:2048
                w = NB if part < 2 else 2 * NB
                c0 = part * NB
                t = xpool.tile([P, NK, w], f16, name=f"xg_{tag}{part}", bufs=1)
                nc.sync.dma_start(
                    t[:], src.rearrange("(k p) s -> p k s", p=P)[:, :, c0:c0 + w])
                gd[part] = t

            def x_ap(gd, k, n):
                if n < 2:
                    return gd[n][:, k, :]
                return gd[2][:, k, (n - 2) * NB:(n - 1) * NB]

            # DMA program order tuned for the critical path:
            # K0(0)/Q0(0) projections first, then V, then the rest.
            load_group(xq_t, xq, 0, "q")
            load_group(xk_t, xk, 0, "k")
            nc.sync.dma_start(wv_sb[:], wvT.rearrange("(k p) d -> p k d", p=P))
            load_group(xv_t, xv, 0, "v")
            load_group(xk_t, xk, 1, "k")
            load_group(xv_t, xv, 1, "v")
            load_group(xk_t, xk, 2, "k")
            load_group(xq_t, xq, 1, "q")
            load_group(xv_t, xv, 2, "v")
            load_group(xq_t, xq, 2, "q")
            nc.sync.dma_start(wo_sb[:], woT.rearrange("(k p) d -> p k d", p=P))

            # constants for vp (ones rows); f32r shares f32's bit layout
            nc.gpsimd.memset(onesrow[:], 1.0)
            for h in range(4):
                nc.gpsimd.memset(vp[:, :, h * (HD + 1) + HD:h * (HD + 1) + HD + 1], 1.0)

            # ---- V projection for one s-tile ----
            def v_block(s):
                n, c = divmod(s, 4)
                ps = ops.tile([P, NB], f32, name="ops_t")
                for k in range(NK):
                    nc.tensor.matmul(
                        ps[:, :DL],
                        x_ap(xv_t, k, n)[:, c * P:(c + 1) * P],
                        wv_sb[:, k, :],
                        start=(k == 0),
                        stop=(k == NK - 1),
                    )
                dst = vp[:, s, 0:4 * (HD + 1)].rearrange(
                    "p (h d) -> p h d", h=4)[:, :, 0:HD]
                nc.vector.tensor_copy(dst, ps[:, :DL].rearrange("p (h d) -> p h d", h=4))

            # ---- K/Q projection, one n-block (woff: 0 = Wq, DL = Wk) ----
            def proj_qk_n(woff, xd, dst, bcol, m, n):
                ps = ops.tile([P, NB], f32, name="ops_t")
                for k in range(NK):
                    nc.tensor.matmul(
                        ps[:],
                        wqk_sb[:, k, woff + m * P:woff + (m + 1) * P],
                        x_ap(xd, k, n),
                        start=(k == 0),
                        stop=(k == NK - 1),
                    )
                nc.vector.tensor_scalar_add(
                    dst[:, m, n * NB:(n + 1) * NB], ps[:],
                    bias_sb[:, bcol + m:bcol + m + 1],
                )

            def wo_block(n, last=False):
                otb = evpool.tile([P, 8, NB], f16, name="otb_t", bufs=1)
                for mo in range(8):
                    ps = ops.tile([P, NB], f32, name="ops_t")
                    for k2 in range(2):
                        nc.tensor.matmul(
                            ps[:],
                            wo_sb[:, k2, mo * P:(mo + 1) * P],
                            onorm[:, k2, n * NB:(n + 1) * NB],
                            start=(k2 == 0),
                            stop=(k2 == 1),
                        )
                    nc.any.tensor_copy(otb[:, mo, :], ps[:])
                    if last:
                        # per-mo output DMAs let the store overlap the
                        # remaining matmuls in the kernel tail
                        nc.sync.dma_start(
                            out[mo * P:(mo + 1) * P, n * NB:(n + 1) * NB],
                            otb[:, mo, :],
                        )
                if not last:
                    nc.sync.dma_start(
                        out.rearrange("(m p) s -> p m s", p=P)[:, :, n * NB:(n + 1) * NB],
                        otb[:],
                    )

            def attention_ib(p, ib, oa, ob, v_inline=False, mids=None):
                isl = slice(ib * NB, (ib + 1) * NB)
                es = []

                def emit_av(jb):
                    e = es[jb]
                    offa = 2 * p * (HD + 1)
                    offb = (2 * p + 1) * (HD + 1)
                    nc.tensor.matmul(
                        oa[0:HD + 1, :], vp[:, jb, offa:offa + HD + 1], e[:, 0:NB],
                        start=(jb == 0), stop=(jb == NJ - 1),
                    )
                    nc.tensor.matmul(
                        ob[0:HD + 1, :], vp[:, jb, offb:offb + HD + 1], e[:, NB:2 * NB],
                        start=(jb == 0), stop=(jb == NJ - 1),
                    )

                # one-stage software pipeline: QK(jb) ahead of AV(jb-1)
                for jb in range(NJ):
                    jsl = slice(jb * P, (jb + 1) * P)
                    sc = scps.tile([P, 2 * NB], f32, name="sc_t")
                    nc.tensor.matmul(
                        sc[:, 0:NB], kpt[0:HD, p, jsl], qpt[0:HD, p, isl],
                        start=True, stop=True,
                    )
                    nc.tensor.matmul(
                        sc[:, NB:2 * NB], kpt[HD:P, p, jsl], qpt[HD:P, p, isl],
                        start=True, stop=True,
                    )
                    e = exppool.tile([P, 2 * NB], f16, name="e_t")
                    nc.scalar.activation(e[:], sc[:], Exp, scale=0.125)
                    es.append(e)
                    if v_inline and 4 <= jb:
                        v_block(jb)
                    if mids is not None and jb in mids:
                        mids[jb]()
                    if jb >= 1:
                        emit_av(jb - 1)
                emit_av(NJ - 1)

            def norm_ib(p, ib, oa, ob, after=None):
                # normalization: l sits at row HD of oa/ob.  Broadcast l
                # along partitions via a K=1 fp16 PE matmul (ops pool, so
                # the score-psum pool never waits on this chain), then
                # 1/l = exp(-ln l) on ScalarE, multiply on DVE.
                isl = slice(ib * NB, (ib + 1) * NB)
                lrow = lrpool.tile([P, 2 * NB], f16, name="lrow_t")
                lnb = lrpool.tile([P, 2 * NB], f32, name="lnb_t")
                rbs = lrpool.tile([P, 2 * NB], f32, name="rbs_t")
                # B-half first: its result goes through the (slow)
                # partition-shift DMA, so start that chain earliest.
                nc.vector.tensor_copy(lrow[HD:HD + 1, NB:2 * NB], ob[HD:HD + 1, :])
                rbb = ops.tile([P, NB], f32, name="ops_t")
                nc.tensor.matmul(
                    rbb[:], onesrow[HD:HD + 1, :], lrow[HD:HD + 1, NB:2 * NB],
                    start=True, stop=True,
                )
                nc.scalar.activation(lnb[:, NB:2 * NB], rbb[:], Ln)
                nc.scalar.activation(
                    rbs[:, NB:2 * NB], lnb[:, NB:2 * NB], Exp, scale=-1.0)
                tmpb = evpool.tile([P, NB], f16, name="tmpb_t")
                nc.vector.tensor_mul(tmpb[0:HD, :], ob[0:HD, :], rbs[0:HD, NB:2 * NB])
                nc.gpsimd.dma_start(onorm[HD:P, p, isl], tmpb[0:HD, :])
                # A-half
                nc.vector.tensor_copy(lrow[HD:HD + 1, 0:NB], oa[HD:HD + 1, :])
                rba = ops.tile([P, NB], f32, name="ops_t")
                nc.tensor.matmul(
                    rba[:], onesrow[HD:HD + 1, :], lrow[HD:HD + 1, 0:NB],
                    start=True, stop=True,
                )
                nc.scalar.activation(lnb[:, 0:NB], rba[:], Ln)
                nc.scalar.activation(rbs[:, 0:NB], lnb[:, 0:NB], Exp, scale=-1.0)
                nc.vector.tensor_mul(onorm[0:HD, p, isl], oa[0:HD, :], rbs[0:HD, 0:NB])
                if after is not None:
                    after()

            # ---- schedule ----
            def P_(woff, xd, dst, bcol, m, n):
                return lambda: proj_qk_n(woff, xd, dst, bcol, m, n)

            K0 = lambda n: P_(DL, xk_t, kpt, 2, 0, n)
            Q0 = lambda n: P_(0, xq_t, qpt, 0, 0, n)
            K1 = lambda n: P_(DL, xk_t, kpt, 2, 1, n)
            Q1 = lambda n: P_(0, xq_t, qpt, 0, 1, n)

            Q0(0)()
            K0(0)()
            for s in range(4):
                v_block(s)

            blocks = [
                (0, 0, True, {2: K0(1), 5: K0(2), 8: K0(3), 11: Q0(1)}, None),
                (0, 1, False, {4: Q0(2), 10: K1(0)}, None),
                (0, 2, False, {4: Q0(3), 10: K1(1)}, None),
                (0, 3, False, {6: Q1(0)}, None),
                (1, 0, False, {1: K1(2), 5: K1(3), 10: Q1(1)}, lambda: wo_block(0)),
                (1, 1, False, {8: Q1(2)}, lambda: wo_block(1)),
                (1, 2, False, {8: Q1(3)}, lambda: wo_block(2)),
                (1, 3, False, None, lambda: wo_block(3, last=True)),
            ]
            for (p, ib, v_inline, mids, after) in blocks:
                oa = oaob.tile([P, NB], f32, name="oa_t")
                ob = oaob.tile([P, NB], f32, name="ob_t")
                attention_ib(p, ib, oa, ob, v_inline=v_inline, mids=mids)
                norm_ib(p, ib, oa, ob, after=after)

    return nc


def _get_nc():
    if "nc" not in _nc_cache:
        _install_bir_fix()
        _nc_cache["nc"] = _build_nc()
    return _nc_cache["nc"]


# --------------------------------------------------------------------------
# Host wrapper
# --------------------------------------------------------------------------
def run(inputs, trace=False):
    from concourse.bass_utils import run_bass_kernel_spmd

    q = np.asarray(inputs["q"], np.float32)
    k = np.asarray(inputs["k"], np.float32)
    v = np.asarray(inputs["v"], np.float32)
    Wq = np.asarray(inputs["Wq"], np.float32)
    bq = np.asarray(inputs["bq"], np.float32)
    Wk = np.asarray(inputs["Wk"], np.float32)
    bk = np.asarray(inputs["bk"], np.float32)
    Wv = np.asarray(inputs["Wv"], np.float32)
    bv = np.asarray(inputs["bv"], np.float32)
    Wo = np.asarray(inputs["Wo"], np.float32)
    bo = np.asarray(inputs["bo"], np.float32)

    nc = _get_nc()

    xT = {}
    for b in range(2):
        xT[b] = (
            np.ascontiguousarray(q[b].T).astype(np.float16),
            np.ascontiguousarray(k[b].T).astype(np.float16),
            np.ascontiguousarray(v[b].T).astype(np.float16),
        )

    in_maps = []
    for c in range(8):
        b, g = divmod(c, 4)
        sl = slice(g * DL, (g + 1) * DL)
        bias = np.stack(
            [bq[sl][:P], bq[sl][P:], bk[sl][:P], bk[sl][P:]], axis=1
        ).astype(np.float32)
        wqk = np.concatenate(
            [Wq[sl, :].T, Wk[sl, :].T], axis=1)  # [H, 2*DL]
        in_maps.append({
            "xq": xT[b][0],
            "xk": xT[b][1],
            "xv": xT[b][2],
            "wqkT": np.ascontiguousarray(wqk).astype(np.float16),
            "wvT": np.ascontiguousarray(Wv[sl, :].T).astype(np.float16),
            "woT": np.ascontiguousarray(Wo[:, sl].T).astype(np.float16),
            "bias": bias,
        })

    res = run_bass_kernel_spmd(
        nc, in_maps, core_ids=list(range(8)), trace=trace,
    )
    outs = [r["out"] for r in res.results]

    const = (Wo @ bv + bo).astype(np.float32)  # [1024]
    full = np.empty((2, S, H), np.float32)
    for b in range(2):
        acc = outs[4 * b].astype(np.float32).copy()
        for g in range(1, 4):
            acc += outs[4 * b + g]
        full[b] = acc.T + const
    return full, res


def kernel(**inputs):
    full, _ = run(inputs, trace=False)
    return full
